# revision 11
# baseline (speedup 1.0000x reference)
"""Trainium2 SPMD kernel for H2OLlama attention (GQA + RoPE + causal softmax + o_proj).

Sharding (8 cores): core = b*4 + g  (b in {0,1} batch, g in {0..3} head group).
Each core handles one batch element, 8 q-heads (g*8..g*8+8) and its 2 kv-heads,
computes QKV projections, RoPE, causal attention, and the row-sharded o_proj
partial product.  Host sums the 4 partials per batch (the o_proj all-reduce)
and transposes back.

All matmuls run in bf16 with fp32 PSUM accumulation; softmax runs in fp32 on
the Scalar engine (exp, no max subtraction -- scores are O(1) here).

Perf notes vs the first working version:
- softmax denominators for the large tiles accumulate on the Vector engine
  (bf16 pair tree) finished by two PSUM row-sum matmuls per (h, qt) instead of
  one ones-matmul per key block -- removes ~55us of PE column streaming; the
  small qt=0 tiles keep the per-block PSUM row-sum (engine balance).
- the 1/r broadcast matmul streams f32r (1 cycle/row) instead of f32 (4).
- scores/exp run on paired [128, 1024] PSUM tiles (half the Act instructions).
- x loads are chunked ([128, 8, 1024] x4) and DMA-issue order puts the first
  chunk + first weight ahead of the constants, so the PE starts ~6us in.
- the last o_proj dout runs column-major so stores overlap the final matmuls.
"""

import math
import sys

import numpy as np

sys.path.insert(0, "/opt/trn_rl_repo")

import ml_dtypes

import concourse.bass as bass
import concourse.tile as tile
from concourse import bacc, mybir
from concourse.bass_utils import run_bass_kernel_spmd

BF16 = mybir.dt.bfloat16
F32 = mybir.dt.float32
F32R = mybir.dt.float32r

HIDDEN = 4096
N_HEADS = 32
N_KV_HEADS = 8
HEAD_DIM = 128
B, S = 2, 2048
ROPE_THETA = 10000.0

N_CORES = 8
HEADS_PER_CORE = N_HEADS // 4  # 8 q heads per core (4 head groups)
KV_PER_CORE = N_KV_HEADS // 4  # 2 kv heads per core
QDIM = HEADS_PER_CORE * HEAD_DIM  # 1024
KVDIM = KV_PER_CORE * HEAD_DIM  # 256
HC = HIDDEN // 128  # 32 hidden chunks
TT512 = S // 512  # 4 token tiles of 512
TT128 = S // 128  # 16 token tiles of 128
SCALE = 1.0 / math.sqrt(HEAD_DIM)

_BUILD_CACHE = {}


def _build_program():
    nc = bacc.Bacc("TRN2", target_bir_lowering=False, debug=False, num_devices=N_CORES)

    # ---- DRAM I/O ----
    xt_d = nc.dram_tensor("xt", [128, HC, S], BF16, kind="ExternalInput")
    wq_d = nc.dram_tensor("wq", [HEADS_PER_CORE, 128, HC, 128], BF16, kind="ExternalInput")
    wk_d = nc.dram_tensor("wk", [KV_PER_CORE, 128, HC, 128], BF16, kind="ExternalInput")
    wv_d = nc.dram_tensor("wv", [128, HC, KVDIM], BF16, kind="ExternalInput")
    wo_d = nc.dram_tensor("wo", [HC, 128, HEADS_PER_CORE, 128], BF16, kind="ExternalInput")
    cos_d = nc.dram_tensor("cosT", [128, S], F32, kind="ExternalInput")
    sin_d = nc.dram_tensor("sinT", [128, S], F32, kind="ExternalInput")
    rt_d = nc.dram_tensor("rt", [128, 128], BF16, kind="ExternalInput")
    mp0_d = nc.dram_tensor("maskp0", [128, 1024], BF16, kind="ExternalInput")
    mp2_d = nc.dram_tensor("maskp2", [128, 1024], BF16, kind="ExternalInput")
    out_d = nc.dram_tensor("out", [HIDDEN, S], BF16, kind="ExternalOutput")

    with tile.TileContext(nc) as tc:
        _kernel_body(nc, tc, xt_d, wq_d, wk_d, wv_d, wo_d, cos_d, sin_d, rt_d, mp0_d, mp2_d, out_d)

    nc.compile()
    return nc


def _kernel_body(nc, tc, xt_d, wq_d, wk_d, wv_d, wo_d, cos_d, sin_d, rt_d, mp0_d, mp2_d, out_d):
    EXP = mybir.ActivationFunctionType.Exp

    with (
        tc.tile_pool(name="qr", bufs=1) as qr_pool,
        tc.tile_pool(name="kr", bufs=1) as kr_pool,
        tc.tile_pool(name="vv", bufs=1) as v_pool,
        tc.tile_pool(name="aa", bufs=1) as a_pool,
        tc.tile_pool(name="consts", bufs=1) as const_pool,
    ):
        # persistent tensors
        qr = [qr_pool.tile([128, S], BF16, tag=f"qr{h}", name=f"qr{h}") for h in range(HEADS_PER_CORE)]
        kr = [kr_pool.tile([128, S], BF16, tag=f"kr{k}", name=f"kr{k}") for k in range(KV_PER_CORE)]
        vt = v_pool.tile([128, TT128, KVDIM], BF16, tag="v", name="vt")
        at = [a_pool.tile([128, S], BF16, tag=f"a{h}", name=f"a{h}") for h in range(HEADS_PER_CORE)]

        cos_sb = const_pool.tile([128, S], F32, tag="cos", name="cos_sb")
        sin_sb = const_pool.tile([128, S], F32, tag="sin", name="sin_sb")
        rt_sb = const_pool.tile([128, 128], BF16, tag="rt", name="rt_sb")
        mp_sb = {
            0: const_pool.tile([128, 1024], BF16, tag="mp0", name="mp0_sb"),
            2: const_pool.tile([128, 1024], BF16, tag="mp2", name="mp2_sb"),
        }
        ones_row = const_pool.tile([1, 128], F32R, tag="ones_row", name="ones_row")

        onesf_row = const_pool.tile([1, 128], F32, tag="onesf_row", name="onesf_row")
        ones_colb = const_pool.tile([128, 1], BF16, tag="ones_colb", name="ones_colb")
        nc.vector.memset(onesf_row[:], 1.0)
        nc.vector.memset(ones_colb[:], 1.0)
        # memset cannot write f32r; convert through a DVE copy instead
        nc.vector.tensor_copy(ones_row[:], onesf_row[:])

        # ================= Stage 1+2: projections + RoPE =================
        with (
            tc.tile_pool(name="xt", bufs=1) as x_pool,
            tc.tile_pool(name="wqk", bufs=2) as wqk_pool,
            tc.tile_pool(name="wv", bufs=1) as wv_pool,
            tc.tile_pool(name="qkraw", bufs=1) as qkraw_pool,
            tc.tile_pool(name="ropetmp", bufs=1) as rtmp_pool,
            tc.tile_pool(name="qkps", bufs=2, space="PSUM") as qk_psum,
            tc.tile_pool(name="rotps", bufs=1, space="PSUM") as rot_psum,
            tc.tile_pool(name="vps", bufs=2, space="PSUM") as v_psum,
        ):
            wv_sb = wv_pool.tile([128, HC, KVDIM], BF16, tag="wv", name="wv_sb")
            for half in range(2):
                toff = half * 1024
                # x chunks for this half: 4 tiles of 8 hidden-chunks each
                xs = []
                for cg in range(4):
                    xtile = x_pool.tile([128, 8, 1024], BF16, tag=f"x{cg}", name=f"x{cg}")
                    xs.append(xtile)

                def xdma(cg, n_sub=2, eng=None):
                    # sub-DMAs per chunk so downstream matmuls can start on
                    # the first hidden-chunks while the rest streams in
                    step = 8 // n_sub
                    for s in range(n_sub):
                        lo, hi = s * step, (s + 1) * step
                        (eng or nc.sync).dma_start(
                            xs[cg][:, lo:hi, :],
                            xt_d.ap()[:, cg * 8 + lo : cg * 8 + hi, toff : toff + 1024],
                        )

                def xsl(hc, lo, sz):
                    return xs[hc // 8][:, hc % 8, lo : lo + sz]

                w_tiles = {}

                def wdma(do, split=False):
                    if do < HEADS_PER_CORE:
                        w_src = wq_d.ap()[do]
                    else:
                        w_src = wk_d.ap()[do - HEADS_PER_CORE]
                    w_sb = wqk_pool.tile([128, HC, 128], BF16, tag="wqk", name="w_sb")
                    if split:
                        # sub-DMAs so the first matmuls start before the whole
                        # weight tile lands
                        for s in range(4):
                            nc.sync.dma_start(
                                w_sb[:, s * 8 : (s + 1) * 8, :], w_src[:, s * 8 : (s + 1) * 8, :]
                            )
                    else:
                        nc.sync.dma_start(w_sb[:], w_src)
                    w_tiles[do] = w_sb

                # DMA issue order: first weight + first x chunk interleaved at
                # fine grain ahead of the constants so the PE starts ~3us in;
                # constants are only needed by RoPE (DVE) / V-proj, much later.
                if half == 0:
                    # interleave the first weight's pieces with the first x
                    # pieces so the very first matmuls have both inputs early
                    w_sb0 = wqk_pool.tile([128, HC, 128], BF16, tag="wqk", name="w_sb")
                    w_src0 = wq_d.ap()[0]
                    for s in range(4):
                        nc.sync.dma_start(
                            w_sb0[:, s * 8 : (s + 1) * 8, :], w_src0[:, s * 8 : (s + 1) * 8, :]
                        )
                        nc.sync.dma_start(
                            xs[0][:, s * 2 : (s + 1) * 2, :],
                            xt_d.ap()[:, s * 2 : (s + 1) * 2, toff : toff + 1024],
                        )
                    w_tiles[0] = w_sb0
                else:
                    xdma(0, n_sub=4)
                    wdma(0)
                xdma(1)
                xdma(2)
                xdma(3)
                wdma(1)
                if half == 0:
                    nc.sync.dma_start(rt_sb[:], rt_d.ap())
                    nc.sync.dma_start(cos_sb[:], cos_d.ap())
                    nc.sync.dma_start(sin_sb[:], sin_d.ap())
                    nc.sync.dma_start(mp_sb[0][:], mp0_d.ap())
                    nc.sync.dma_start(mp_sb[2][:], mp2_d.ap())
                    nc.sync.dma_start(wv_sb[:], wv_d.ap())

                # ---- Q^T and K^T projections (+ RoPE) ----
                for do in range(HEADS_PER_CORE + KV_PER_CORE):  # 8 q douts, 2 k douts
                    if do + 2 < HEADS_PER_CORE + KV_PER_CORE:
                        wdma(do + 2)
                    if do < HEADS_PER_CORE:
                        dst = qr[do]
                    else:
                        dst = kr[do - HEADS_PER_CORE]
                    w_sb = w_tiles.pop(do)

                    ps = qk_psum.tile([128, 1024], F32, tag="qk", name="qkps")
                    for hc in range(HC):
                        for tt in range(2):
                            nc.tensor.matmul(
                                ps[:, tt * 512 : (tt + 1) * 512],
                                lhsT=w_sb[:, hc, :],
                                rhs=xsl(hc, tt * 512, 512),
                                start=(hc == 0),
                                stop=(hc == HC - 1),
                            )
                    # RoPE on the full 1024-token tile
                    raw = qkraw_pool.tile([128, 1024], BF16, tag="raw", name="raw")
                    nc.scalar.copy(raw[:], ps[:])
                    rot = rot_psum.tile([128, 1024], F32, tag="rot", name="rot")
                    for tt in range(2):
                        nc.tensor.matmul(
                            rot[:, tt * 512 : (tt + 1) * 512],
                            lhsT=rt_sb[:],
                            rhs=raw[:, tt * 512 : (tt + 1) * 512],
                            start=True,
                            stop=True,
                        )
                    t1 = rtmp_pool.tile([128, 1024], F32, tag="t1", name="t1")
                    nc.vector.tensor_mul(t1[:], raw[:], cos_sb[:, toff : toff + 1024])
                    t2 = rtmp_pool.tile([128, 1024], F32, tag="t2", name="t2")
                    nc.vector.tensor_mul(t2[:], rot[:], sin_sb[:, toff : toff + 1024])
                    nc.vector.tensor_add(dst[:, toff : toff + 1024], t1[:], t2[:])

                # ---- V projection (natural [t, d] layout) ----
                for t8 in range(8):
                    j = half * 8 + t8
                    ps = v_psum.tile([128, KVDIM], F32, tag="vps", name="vps")
                    for hc in range(HC):
                        nc.tensor.matmul(
                            ps[:],
                            lhsT=xsl(hc, t8 * 128, 128),
                            rhs=wv_sb[:, hc, :],
                            start=(hc == 0),
                            stop=(hc == HC - 1),
                        )
                    nc.scalar.copy(vt[:, j, :], ps[:])

        # ================= Stage 3: attention =================
        with (
            tc.tile_pool(name="pp", bufs=4) as p_pool,
            tc.tile_pool(name="acc", bufs=2) as acc_pool,
            tc.tile_pool(name="rinv", bufs=2) as rinv_pool,
            tc.tile_pool(name="sps", bufs=2, space="PSUM") as s_psum,
            tc.tile_pool(name="ops", bufs=2, space="PSUM") as o_psum,
            tc.tile_pool(name="rps", bufs=1, space="PSUM") as r_psum,
            tc.tile_pool(name="bps", bufs=1, space="PSUM") as b_psum,
        ):
            for h in range(HEADS_PER_CORE):
                kvl = h // 4
                for qt in range(TT512):
                    nj = 4 * qt + 4
                    npair = nj // 2
                    # small tiles keep the baseline PE-PSUM row-sum; large tiles
                    # accumulate denominators on the DVE to unload the PE.  The
                    # split keeps PE, DVE, and Act roughly level in this stage.
                    dve_acc = qt >= 1
                    q_rhs = qr[h][:, qt * 512 : (qt + 1) * 512]
                    o_ps = o_psum.tile([128, 512], F32, tag="o", name="o_ps")
                    r_ps = r_psum.tile([1, 512], F32, tag="r", name="r_ps")
                    if dve_acc:
                        acc2 = acc_pool.tile([128, 1024], BF16, tag="acc2", name="acc2")

                    pts = {}

                    def emit_pair(pi):
                        j0 = 2 * pi
                        s_ps = s_psum.tile([128, 1024], F32, tag="s", name="s_ps")
                        for u in range(2):
                            nc.tensor.matmul(
                                s_ps[:, u * 512 : (u + 1) * 512],
                                lhsT=kr[kvl][:, (j0 + u) * 128 : (j0 + u + 1) * 128],
                                rhs=q_rhs,
                                start=True,
                                stop=True,
                            )
                        p2 = p_pool.tile([128, 1024], BF16, tag="p", name="p2")
                        nc.scalar.activation(p2[:], s_ps[:], EXP, scale=SCALE)
                        if j0 >= 4 * qt:  # diagonal pair: multiplicative causal mask
                            a = j0 - 4 * qt  # 0 or 2
                            nc.vector.tensor_mul(p2[:], p2[:], mp_sb[a][:])
                        pts[pi] = p2

                    # software pipeline: score pairs run 2 iterations ahead of AV
                    emit_pair(0)
                    if npair > 1:
                        emit_pair(1)
                    for pi in range(npair):
                        if pi + 2 < npair:
                            emit_pair(pi + 2)
                        p2 = pts.pop(pi)
                        if dve_acc:
                            # denominator accumulation on DVE (pair-wide bf16 tree)
                            if pi == 0:
                                nc.vector.tensor_copy(acc2[:], p2[:])
                            else:
                                nc.vector.tensor_add(acc2[:], acc2[:], p2[:])
                        for u in range(2):
                            j = 2 * pi + u
                            nc.tensor.matmul(
                                o_ps[:],
                                lhsT=vt[:, j, kvl * 128 : (kvl + 1) * 128],
                                rhs=p2[:, u * 512 : (u + 1) * 512],
                                start=(j == 0),
                                stop=(j == nj - 1),
                            )
                            if not dve_acc:
                                nc.tensor.matmul(
                                    r_ps[:],
                                    lhsT=ones_colb[:],
                                    rhs=p2[:, u * 512 : (u + 1) * 512],
                                    start=(j == 0),
                                    stop=(j == nj - 1),
                                )
                    if dve_acc:
                        # row-sum both acc2 halves directly on PE (bf16,
                        # PSUM-accumulated) -- no DVE fold on the critical path
                        for u in range(2):
                            nc.tensor.matmul(
                                r_ps[:],
                                lhsT=ones_colb[:],
                                rhs=acc2[:, u * 512 : (u + 1) * 512],
                                start=(u == 0),
                                stop=(u == 1),
                            )
                    rinv = rinv_pool.tile([1, 512], F32R, tag="rinv", name="rinv")
                    with nc.allow_low_precision(reason="f32r reciprocal feeds the f32r broadcast matmul; ~10-bit mantissa is ample for softmax denominators"):
                        nc.vector.reciprocal(rinv[:], r_ps[:])
                    b_ps = b_psum.tile([128, 512], F32, tag="b", name="b_ps")
                    nc.tensor.matmul(
                        b_ps[:],
                        lhsT=ones_row[:],
                        rhs=rinv[:],
                        start=True,
                        stop=True,
                    )
                    # stage the broadcast through SBUF so the normalize multiply
                    # only touches one PSUM operand; alternate the copy engine
                    # to keep Act and DVE level (both near-saturated here)
                    b_sb = rinv_pool.tile([128, 512], F32, tag="bsb", name="b_sb")
                    if qt % 2 == 1:
                        nc.scalar.copy(b_sb[:], b_ps[:])
                    else:
                        nc.vector.tensor_copy(b_sb[:], b_ps[:])
                    nc.vector.tensor_mul(at[h][:, qt * 512 : (qt + 1) * 512], o_ps[:], b_sb[:])

        # ================= Stage 4: o_proj (out^T layout) =================
        with (
            tc.tile_pool(name="wo", bufs=2) as wo_pool,
            tc.tile_pool(name="oout", bufs=4) as out_pool,
            tc.tile_pool(name="outps", bufs=2, space="PSUM") as out_psum,
        ):
            for do in range(HC):  # 32 dout tiles of 128
                wo_sb = wo_pool.tile([128, HEADS_PER_CORE, 128], BF16, tag="wo", name="wo_sb")
                nc.sync.dma_start(wo_sb[:], wo_d.ap()[do])
                pss = [out_psum.tile([128, 512], F32, tag=f"op{tt}", name=f"op{tt}") for tt in range(TT512)]

                def store(tt):
                    ot = out_pool.tile([128, 512], BF16, tag="ot", name="ot")
                    nc.vector.tensor_copy(ot[:], pss[tt][:])
                    nc.sync.dma_start(
                        out_d.ap()[do * 128 : (do + 1) * 128, tt * 512 : (tt + 1) * 512], ot[:]
                    )

                if do < HC - 1:
                    for a in range(HEADS_PER_CORE):
                        for tt in range(TT512):
                            nc.tensor.matmul(
                                pss[tt][:],
                                lhsT=wo_sb[:, a, :],
                                rhs=at[a][:, tt * 512 : (tt + 1) * 512],
                                start=(a == 0),
                                stop=(a == HEADS_PER_CORE - 1),
                            )
                    for tt in range(TT512):
                        store(tt)
                else:
                    # last dout: tt-major so each column block stores while the
                    # next one is still accumulating (shorter end tail)
                    for tt in range(TT512):
                        for a in range(HEADS_PER_CORE):
                            nc.tensor.matmul(
                                pss[tt][:],
                                lhsT=wo_sb[:, a, :],
                                rhs=at[a][:, tt * 512 : (tt + 1) * 512],
                                start=(a == 0),
                                stop=(a == HEADS_PER_CORE - 1),
                            )
                        store(tt)


# ======================= host-side sharding =======================


def _rope_tables(position_ids_b):
    pos = position_ids_b.astype(np.float32)  # [S]
    inv_freq = 1.0 / (ROPE_THETA ** (np.arange(0, HEAD_DIM, 2, dtype=np.float32) / HEAD_DIM))
    freqs = pos[:, None] * inv_freq[None, :]  # [S, 64]
    emb = np.concatenate([freqs, freqs], axis=1)  # [S, 128]
    cosT = np.ascontiguousarray(np.cos(emb).T.astype(np.float32))  # [128, S]
    sinT = np.ascontiguousarray(np.sin(emb).T.astype(np.float32))
    return cosT, sinT


def _shared_consts():
    rt = np.zeros((128, 128), dtype=ml_dtypes.bfloat16)
    idx = np.arange(64)
    rt[idx, idx + 64] = 1.0  # RT[j, j+64] = +1  (j < 64)
    rt[idx + 64, idx] = -1.0  # RT[j+64, j] = -1
    # pair masks: [128, 1024] where cols [0:512] use block alignment a and
    # cols [512:1024] use alignment a+1; mask[k, q] = (q >= a*128 + k)
    k = np.arange(128)[:, None]
    q = np.arange(512)[None, :]
    maskp = {}
    for a in (0, 2):
        m = np.zeros((128, 1024), dtype=ml_dtypes.bfloat16)
        m[:, 0:512] = (q >= a * 128 + k).astype(ml_dtypes.bfloat16)
        m[:, 512:1024] = (q >= (a + 1) * 128 + k).astype(ml_dtypes.bfloat16)
        maskp[a] = m
    return rt, maskp


def kernel(hidden_states, position_ids, Wq, Wk, Wv, Wo):
    bf16 = ml_dtypes.bfloat16
    if "nc" not in _BUILD_CACHE:
        _BUILD_CACHE["nc"] = _build_program()
    nc = _BUILD_CACHE["nc"]

    rt, maskp = _shared_consts()
    Wq16, Wk16, Wv16, Wo16 = (w.astype(bf16) for w in (Wq, Wk, Wv, Wo))

    xts, coss, sins = [], [], []
    for b in range(B):
        xb = np.asarray(hidden_states[b], dtype=np.float32).T.astype(bf16)  # [4096, S]
        xt = np.ascontiguousarray(xb.reshape(HC, 128, S).transpose(1, 0, 2))  # [128, 32, S]
        xts.append(xt)
        cosT, sinT = _rope_tables(np.asarray(position_ids[b]))
        coss.append(cosT)
        sins.append(sinT)

    in_maps = []
    for core in range(N_CORES):
        b, g = core // 4, core % 4
        wq = np.ascontiguousarray(
            Wq16[:, g * QDIM : (g + 1) * QDIM].reshape(HC, 128, HEADS_PER_CORE, 128).transpose(2, 1, 0, 3)
        )
        wk = np.ascontiguousarray(
            Wk16[:, g * KVDIM : (g + 1) * KVDIM].reshape(HC, 128, KV_PER_CORE, 128).transpose(2, 1, 0, 3)
        )
        wv = np.ascontiguousarray(
            Wv16[:, g * KVDIM : (g + 1) * KVDIM].reshape(HC, 128, KVDIM).transpose(1, 0, 2)
        )
        wo = np.ascontiguousarray(
            Wo16[g * QDIM : (g + 1) * QDIM, :].reshape(HEADS_PER_CORE, 128, HC, 128).transpose(2, 1, 0, 3)
        )
        in_maps.append(
            {
                "xt": xts[b],
                "wq": wq,
                "wk": wk,
                "wv": wv,
                "wo": wo,
                "cosT": coss[b],
                "sinT": sins[b],
                "rt": rt,
                "maskp0": maskp[0],
                "maskp2": maskp[2],
            }
        )

    res = run_bass_kernel_spmd(nc, in_maps, list(range(N_CORES))).results

    out = np.empty((B, S, HIDDEN), dtype=np.float32)
    for b in range(B):
        acc = res[4 * b]["out"].astype(np.float32)
        for g in range(1, 4):
            acc = acc + res[4 * b + g]["out"]
        out[b] = acc.T
    return out


# revision 12
# speedup vs baseline: 1.3366x; 1.3366x over previous
"""Trainium2 SPMD kernel for H2OLlama attention (GQA + RoPE + causal softmax + o_proj).

Sharding (8 cores): core = b*4 + g  (b in {0,1} batch, g in {0..3} head group).
Each core handles one batch element, 8 q-heads (g*8..g*8+8) and its 2 kv-heads,
computes QKV projections, RoPE, causal attention, and the row-sharded o_proj
partial product.  Host sums the 4 partials per batch (the o_proj all-reduce)
and transposes back.

All matmuls run in bf16 with fp32 PSUM accumulation; softmax runs in fp32 on
the Scalar engine (exp, no max subtraction -- scores are O(1) here).

Perf notes vs the first working version:
- softmax denominators for the large tiles accumulate on the Vector engine
  (bf16 pair tree) finished by two PSUM row-sum matmuls per (h, qt) instead of
  one ones-matmul per key block -- removes ~55us of PE column streaming; the
  small qt=0 tiles keep the per-block PSUM row-sum (engine balance).
- the 1/r broadcast matmul streams f32r (1 cycle/row) instead of f32 (4).
- scores/exp run on paired [128, 1024] PSUM tiles (half the Act instructions).
- x loads are chunked ([128, 8, 1024] x4) and DMA-issue order puts the first
  chunk + first weight ahead of the constants, so the PE starts ~6us in.
- the last o_proj dout runs column-major so stores overlap the final matmuls.
"""

import math
import sys

import numpy as np

sys.path.insert(0, "/opt/trn_rl_repo")

import ml_dtypes

import concourse.bass as bass
import concourse.tile as tile
from concourse import bacc, mybir
from concourse.bass_utils import run_bass_kernel_spmd

BF16 = mybir.dt.bfloat16
F32 = mybir.dt.float32
F32R = mybir.dt.float32r

HIDDEN = 4096
N_HEADS = 32
N_KV_HEADS = 8
HEAD_DIM = 128
B, S = 2, 2048
ROPE_THETA = 10000.0

N_CORES = 8
HEADS_PER_CORE = N_HEADS // 4  # 8 q heads per core (4 head groups)
KV_PER_CORE = N_KV_HEADS // 4  # 2 kv heads per core
QDIM = HEADS_PER_CORE * HEAD_DIM  # 1024
KVDIM = KV_PER_CORE * HEAD_DIM  # 256
HC = HIDDEN // 128  # 32 hidden chunks
TT512 = S // 512  # 4 token tiles of 512
TT128 = S // 128  # 16 token tiles of 128
SCALE = 1.0 / math.sqrt(HEAD_DIM)

_BUILD_CACHE = {}


def _build_program():
    nc = bacc.Bacc("TRN2", target_bir_lowering=False, debug=False, num_devices=N_CORES)

    # ---- DRAM I/O ----
    xt_d = nc.dram_tensor("xt", [128, HC, S], BF16, kind="ExternalInput")
    wq_d = nc.dram_tensor("wq", [HEADS_PER_CORE, 128, HC, 128], BF16, kind="ExternalInput")
    wk_d = nc.dram_tensor("wk", [KV_PER_CORE, 128, HC, 128], BF16, kind="ExternalInput")
    wv_d = nc.dram_tensor("wv", [128, HC, KVDIM], BF16, kind="ExternalInput")
    wo_d = nc.dram_tensor("wo", [HC, 128, HEADS_PER_CORE, 128], BF16, kind="ExternalInput")
    cos_d = nc.dram_tensor("cosT", [128, S], F32, kind="ExternalInput")
    sin_d = nc.dram_tensor("sinT", [128, S], F32, kind="ExternalInput")
    rt_d = nc.dram_tensor("rt", [128, 128], BF16, kind="ExternalInput")
    mp0_d = nc.dram_tensor("maskp0", [128, 1024], BF16, kind="ExternalInput")
    mp2_d = nc.dram_tensor("maskp2", [128, 1024], BF16, kind="ExternalInput")
    out_d = nc.dram_tensor("out", [HIDDEN, S], BF16, kind="ExternalOutput")

    with tile.TileContext(nc) as tc:
        _kernel_body(nc, tc, xt_d, wq_d, wk_d, wv_d, wo_d, cos_d, sin_d, rt_d, mp0_d, mp2_d, out_d)

    nc.compile()
    return nc


def _kernel_body(nc, tc, xt_d, wq_d, wk_d, wv_d, wo_d, cos_d, sin_d, rt_d, mp0_d, mp2_d, out_d):
    EXP = mybir.ActivationFunctionType.Exp

    with (
        tc.tile_pool(name="qr", bufs=1) as qr_pool,
        tc.tile_pool(name="kr", bufs=1) as kr_pool,
        tc.tile_pool(name="vv", bufs=1) as v_pool,
        tc.tile_pool(name="aa", bufs=1) as a_pool,
        tc.tile_pool(name="consts", bufs=1) as const_pool,
    ):
        # persistent tensors
        qr = [qr_pool.tile([128, S], BF16, tag=f"qr{h}", name=f"qr{h}") for h in range(HEADS_PER_CORE)]
        kr = [kr_pool.tile([128, S], BF16, tag=f"kr{k}", name=f"kr{k}") for k in range(KV_PER_CORE)]
        vt = v_pool.tile([128, TT128, KVDIM], BF16, tag="v", name="vt")
        at = [a_pool.tile([128, S], BF16, tag=f"a{h}", name=f"a{h}") for h in range(HEADS_PER_CORE)]

        cos_sb = const_pool.tile([128, S], F32, tag="cos", name="cos_sb")
        sin_sb = const_pool.tile([128, S], F32, tag="sin", name="sin_sb")
        rt_sb = const_pool.tile([128, 128], BF16, tag="rt", name="rt_sb")
        mp_sb = {
            0: const_pool.tile([128, 1024], BF16, tag="mp0", name="mp0_sb"),
            2: const_pool.tile([128, 1024], BF16, tag="mp2", name="mp2_sb"),
        }
        ones_row = const_pool.tile([1, 128], F32R, tag="ones_row", name="ones_row")

        onesf_row = const_pool.tile([1, 128], F32, tag="onesf_row", name="onesf_row")
        ones_colb = const_pool.tile([128, 1], BF16, tag="ones_colb", name="ones_colb")
        nc.vector.memset(onesf_row[:], 1.0)
        nc.vector.memset(ones_colb[:], 1.0)
        # memset cannot write f32r; convert through a DVE copy instead
        nc.vector.tensor_copy(ones_row[:], onesf_row[:])

        # ================= Stage 1+2: projections + RoPE =================
        with (
            tc.tile_pool(name="xt", bufs=1) as x_pool,
            tc.tile_pool(name="wqk", bufs=2) as wqk_pool,
            tc.tile_pool(name="wv", bufs=1) as wv_pool,
            tc.tile_pool(name="qkraw", bufs=1) as qkraw_pool,
            tc.tile_pool(name="ropetmp", bufs=1) as rtmp_pool,
            tc.tile_pool(name="qkps", bufs=2, space="PSUM") as qk_psum,
            tc.tile_pool(name="rotps", bufs=1, space="PSUM") as rot_psum,
            tc.tile_pool(name="vps", bufs=2, space="PSUM") as v_psum,
        ):
            wv_sb = wv_pool.tile([128, HC, KVDIM], BF16, tag="wv", name="wv_sb")
            for half in range(2):
                toff = half * 1024
                # x chunks for this half: 4 tiles of 8 hidden-chunks each
                xs = []
                for cg in range(4):
                    xtile = x_pool.tile([128, 8, 1024], BF16, tag=f"x{cg}", name=f"x{cg}")
                    xs.append(xtile)

                def xdma(cg, n_sub=2, eng=None):
                    # sub-DMAs per chunk so downstream matmuls can start on
                    # the first hidden-chunks while the rest streams in
                    step = 8 // n_sub
                    for s in range(n_sub):
                        lo, hi = s * step, (s + 1) * step
                        (eng or nc.sync).dma_start(
                            xs[cg][:, lo:hi, :],
                            xt_d.ap()[:, cg * 8 + lo : cg * 8 + hi, toff : toff + 1024],
                        )

                def xsl(hc, lo, sz):
                    return xs[hc // 8][:, hc % 8, lo : lo + sz]

                w_tiles = {}

                def wdma(do, split=False):
                    if do < HEADS_PER_CORE:
                        w_src = wq_d.ap()[do]
                    else:
                        w_src = wk_d.ap()[do - HEADS_PER_CORE]
                    w_sb = wqk_pool.tile([128, HC, 128], BF16, tag="wqk", name="w_sb")
                    if split:
                        # sub-DMAs so the first matmuls start before the whole
                        # weight tile lands
                        for s in range(4):
                            nc.sync.dma_start(
                                w_sb[:, s * 8 : (s + 1) * 8, :], w_src[:, s * 8 : (s + 1) * 8, :]
                            )
                    else:
                        nc.sync.dma_start(w_sb[:], w_src)
                    w_tiles[do] = w_sb

                # DMA issue order: first weight + first x chunk interleaved at
                # fine grain ahead of the constants so the PE starts ~3us in;
                # constants are only needed by RoPE (DVE) / V-proj, much later.
                if half == 0:
                    # interleave the first weight's pieces with the first x
                    # pieces so the very first matmuls have both inputs early
                    w_sb0 = wqk_pool.tile([128, HC, 128], BF16, tag="wqk", name="w_sb")
                    w_src0 = wq_d.ap()[0]
                    for s in range(4):
                        nc.sync.dma_start(
                            w_sb0[:, s * 8 : (s + 1) * 8, :], w_src0[:, s * 8 : (s + 1) * 8, :]
                        )
                        nc.sync.dma_start(
                            xs[0][:, s * 2 : (s + 1) * 2, :],
                            xt_d.ap()[:, s * 2 : (s + 1) * 2, toff : toff + 1024],
                        )
                    w_tiles[0] = w_sb0
                else:
                    xdma(0, n_sub=4)
                    wdma(0)
                xdma(1)
                xdma(2)
                xdma(3)
                wdma(1)
                if half == 0:
                    nc.sync.dma_start(rt_sb[:], rt_d.ap())
                    nc.sync.dma_start(cos_sb[:], cos_d.ap())
                    nc.sync.dma_start(sin_sb[:], sin_d.ap())
                    nc.sync.dma_start(mp_sb[0][:], mp0_d.ap())
                    nc.sync.dma_start(mp_sb[2][:], mp2_d.ap())
                    nc.sync.dma_start(wv_sb[:], wv_d.ap())

                # ---- Q^T and K^T projections (+ RoPE) ----
                for do in range(HEADS_PER_CORE + KV_PER_CORE):  # 8 q douts, 2 k douts
                    if do + 2 < HEADS_PER_CORE + KV_PER_CORE:
                        wdma(do + 2)
                    if do < HEADS_PER_CORE:
                        dst = qr[do]
                    else:
                        dst = kr[do - HEADS_PER_CORE]
                    w_sb = w_tiles.pop(do)

                    ps = qk_psum.tile([128, 1024], F32, tag="qk", name="qkps")
                    for hc in range(HC):
                        for tt in range(2):
                            nc.tensor.matmul(
                                ps[:, tt * 512 : (tt + 1) * 512],
                                lhsT=w_sb[:, hc, :],
                                rhs=xsl(hc, tt * 512, 512),
                                start=(hc == 0),
                                stop=(hc == HC - 1),
                            )
                    # RoPE on the full 1024-token tile
                    raw = qkraw_pool.tile([128, 1024], BF16, tag="raw", name="raw")
                    nc.scalar.copy(raw[:], ps[:])
                    rot = rot_psum.tile([128, 1024], F32, tag="rot", name="rot")
                    for tt in range(2):
                        nc.tensor.matmul(
                            rot[:, tt * 512 : (tt + 1) * 512],
                            lhsT=rt_sb[:],
                            rhs=raw[:, tt * 512 : (tt + 1) * 512],
                            start=True,
                            stop=True,
                        )
                    t1 = rtmp_pool.tile([128, 1024], F32, tag="t1", name="t1")
                    nc.vector.tensor_mul(t1[:], raw[:], cos_sb[:, toff : toff + 1024])
                    t2 = rtmp_pool.tile([128, 1024], F32, tag="t2", name="t2")
                    nc.vector.tensor_mul(t2[:], rot[:], sin_sb[:, toff : toff + 1024])
                    nc.vector.tensor_add(dst[:, toff : toff + 1024], t1[:], t2[:])

                # ---- V projection (natural [t, d] layout) ----
                for t8 in range(8):
                    j = half * 8 + t8
                    ps = v_psum.tile([128, KVDIM], F32, tag="vps", name="vps")
                    for hc in range(HC):
                        nc.tensor.matmul(
                            ps[:],
                            lhsT=xsl(hc, t8 * 128, 128),
                            rhs=wv_sb[:, hc, :],
                            start=(hc == 0),
                            stop=(hc == HC - 1),
                        )
                    nc.scalar.copy(vt[:, j, :], ps[:])

        # ================= Stage 3: attention =================
        with (
            tc.tile_pool(name="pp", bufs=4) as p_pool,
            tc.tile_pool(name="acc", bufs=2) as acc_pool,
            tc.tile_pool(name="rinv", bufs=2) as rinv_pool,
            tc.tile_pool(name="sps", bufs=2, space="PSUM") as s_psum,
            tc.tile_pool(name="ops", bufs=2, space="PSUM") as o_psum,
            tc.tile_pool(name="rps", bufs=1, space="PSUM") as r_psum,
            tc.tile_pool(name="bps", bufs=1, space="PSUM") as b_psum,
        ):
            for h in range(HEADS_PER_CORE):
                kvl = h // 4
                for qt in range(TT512):
                    nj = 4 * qt + 4
                    npair = nj // 2
                    # small tiles keep the baseline PE-PSUM row-sum; large tiles
                    # accumulate denominators on the DVE to unload the PE.  The
                    # split keeps PE, DVE, and Act roughly level in this stage.
                    dve_acc = qt >= 1
                    q_rhs = qr[h][:, qt * 512 : (qt + 1) * 512]
                    o_ps = o_psum.tile([128, 512], F32, tag="o", name="o_ps")
                    r_ps = r_psum.tile([1, 512], F32, tag="r", name="r_ps")
                    if dve_acc:
                        acc2 = acc_pool.tile([128, 1024], BF16, tag="acc2", name="acc2")

                    pts = {}

                    def emit_pair(pi):
                        j0 = 2 * pi
                        s_ps = s_psum.tile([128, 1024], F32, tag="s", name="s_ps")
                        trimmed = dve_acc and j0 - 4 * qt == 2
                        if trimmed:
                            # last diagonal pair (blocks a=2,3): queries below
                            # the causal boundary are fully masked -- compute
                            # only the valid column ranges [256:512] / [896:1024]
                            nc.tensor.matmul(
                                s_ps[:, 256:512],
                                lhsT=kr[kvl][:, j0 * 128 : (j0 + 1) * 128],
                                rhs=q_rhs[:, 256:512],
                                start=True,
                                stop=True,
                            )
                            nc.tensor.matmul(
                                s_ps[:, 896:1024],
                                lhsT=kr[kvl][:, (j0 + 1) * 128 : (j0 + 2) * 128],
                                rhs=q_rhs[:, 384:512],
                                start=True,
                                stop=True,
                            )
                            p2 = p_pool.tile([128, 1024], BF16, tag="p", name="p2")
                            nc.scalar.activation(p2[:, 256:512], s_ps[:, 256:512], EXP, scale=SCALE)
                            nc.scalar.activation(p2[:, 896:1024], s_ps[:, 896:1024], EXP, scale=SCALE)
                            nc.vector.tensor_mul(p2[:, 256:512], p2[:, 256:512], mp_sb[2][:, 256:512])
                            nc.vector.tensor_mul(p2[:, 896:1024], p2[:, 896:1024], mp_sb[2][:, 896:1024])
                            pts[pi] = p2
                            return
                        for u in range(2):
                            nc.tensor.matmul(
                                s_ps[:, u * 512 : (u + 1) * 512],
                                lhsT=kr[kvl][:, (j0 + u) * 128 : (j0 + u + 1) * 128],
                                rhs=q_rhs,
                                start=True,
                                stop=True,
                            )
                        p2 = p_pool.tile([128, 1024], BF16, tag="p", name="p2")
                        nc.scalar.activation(p2[:], s_ps[:], EXP, scale=SCALE)
                        if j0 >= 4 * qt:  # diagonal pair: multiplicative causal mask
                            a = j0 - 4 * qt  # 0 or 2
                            nc.vector.tensor_mul(p2[:], p2[:], mp_sb[a][:])
                        pts[pi] = p2

                    # software pipeline: score pairs run 2 iterations ahead of AV
                    emit_pair(0)
                    if npair > 1:
                        emit_pair(1)
                    for pi in range(npair):
                        if pi + 2 < npair:
                            emit_pair(pi + 2)
                        p2 = pts.pop(pi)
                        trimmed = dve_acc and 2 * pi - 4 * qt == 2
                        if dve_acc:
                            # denominator accumulation on DVE (pair-wide bf16 tree)
                            if pi == 0:
                                nc.vector.tensor_copy(acc2[:], p2[:])
                            elif trimmed:
                                nc.vector.tensor_add(acc2[:, 256:512], acc2[:, 256:512], p2[:, 256:512])
                                nc.vector.tensor_add(acc2[:, 896:1024], acc2[:, 896:1024], p2[:, 896:1024])
                            else:
                                nc.vector.tensor_add(acc2[:], acc2[:], p2[:])
                        for u in range(2):
                            j = 2 * pi + u
                            a = j - 4 * qt
                            if dve_acc and a >= 1:
                                # column-range accumulation stops: cols [0:256]
                                # finish at a=1, [256:384] at a=2, [384:512] at a=3
                                vt_l = vt[:, j, kvl * 128 : (kvl + 1) * 128]
                                if a == 1:
                                    nc.tensor.matmul(o_ps[:, 0:256], lhsT=vt_l,
                                                     rhs=p2[:, 512:768], start=False, stop=True,
                                                     skip_group_check=True)
                                    nc.tensor.matmul(o_ps[:, 256:512], lhsT=vt_l,
                                                     rhs=p2[:, 768:1024], start=False, stop=False,
                                                     skip_group_check=True)
                                elif a == 2:
                                    nc.tensor.matmul(o_ps[:, 256:384], lhsT=vt_l,
                                                     rhs=p2[:, 256:384], start=False, stop=True,
                                                     skip_group_check=True)
                                    nc.tensor.matmul(o_ps[:, 384:512], lhsT=vt_l,
                                                     rhs=p2[:, 384:512], start=False, stop=False,
                                                     skip_group_check=True)
                                else:  # a == 3
                                    nc.tensor.matmul(o_ps[:, 384:512], lhsT=vt_l,
                                                     rhs=p2[:, 896:1024], start=False, stop=True,
                                                     skip_group_check=True)
                                continue
                            nc.tensor.matmul(
                                o_ps[:],
                                lhsT=vt[:, j, kvl * 128 : (kvl + 1) * 128],
                                rhs=p2[:, u * 512 : (u + 1) * 512],
                                start=(j == 0),
                                stop=(not dve_acc and j == nj - 1),
                                skip_group_check=dve_acc,
                            )
                            if not dve_acc:
                                nc.tensor.matmul(
                                    r_ps[:],
                                    lhsT=ones_colb[:],
                                    rhs=p2[:, u * 512 : (u + 1) * 512],
                                    start=(j == 0),
                                    stop=(j == nj - 1),
                                )
                    if dve_acc:
                        # row-sum both acc2 halves directly on PE (bf16,
                        # PSUM-accumulated) -- no DVE fold on the critical path
                        for u in range(2):
                            nc.tensor.matmul(
                                r_ps[:],
                                lhsT=ones_colb[:],
                                rhs=acc2[:, u * 512 : (u + 1) * 512],
                                start=(u == 0),
                                stop=(u == 1),
                            )
                    rinv = rinv_pool.tile([1, 512], F32R, tag="rinv", name="rinv")
                    with nc.allow_low_precision(reason="f32r reciprocal feeds the f32r broadcast matmul; ~10-bit mantissa is ample for softmax denominators"):
                        nc.vector.reciprocal(rinv[:], r_ps[:])
                    b_ps = b_psum.tile([128, 512], F32, tag="b", name="b_ps")
                    nc.tensor.matmul(
                        b_ps[:],
                        lhsT=ones_row[:],
                        rhs=rinv[:],
                        start=True,
                        stop=True,
                    )
                    # stage the broadcast through SBUF so the normalize multiply
                    # only touches one PSUM operand; alternate the copy engine
                    # to keep Act and DVE level (both near-saturated here)
                    b_sb = rinv_pool.tile([128, 512], F32, tag="bsb", name="b_sb")
                    if qt % 2 == 1:
                        nc.scalar.copy(b_sb[:], b_ps[:])
                    else:
                        nc.vector.tensor_copy(b_sb[:], b_ps[:])
                    nc.vector.tensor_mul(at[h][:, qt * 512 : (qt + 1) * 512], o_ps[:], b_sb[:])

        # ================= Stage 4: o_proj (out^T layout) =================
        with (
            tc.tile_pool(name="wo", bufs=2) as wo_pool,
            tc.tile_pool(name="oout", bufs=4) as out_pool,
            tc.tile_pool(name="outps", bufs=2, space="PSUM") as out_psum,
        ):
            for do in range(HC):  # 32 dout tiles of 128
                wo_sb = wo_pool.tile([128, HEADS_PER_CORE, 128], BF16, tag="wo", name="wo_sb")
                nc.sync.dma_start(wo_sb[:], wo_d.ap()[do])
                pss = [out_psum.tile([128, 512], F32, tag=f"op{tt}", name=f"op{tt}") for tt in range(TT512)]

                def store(tt):
                    ot = out_pool.tile([128, 512], BF16, tag="ot", name="ot")
                    nc.vector.tensor_copy(ot[:], pss[tt][:])
                    nc.sync.dma_start(
                        out_d.ap()[do * 128 : (do + 1) * 128, tt * 512 : (tt + 1) * 512], ot[:]
                    )

                if do < HC - 1:
                    for a in range(HEADS_PER_CORE):
                        for tt in range(TT512):
                            nc.tensor.matmul(
                                pss[tt][:],
                                lhsT=wo_sb[:, a, :],
                                rhs=at[a][:, tt * 512 : (tt + 1) * 512],
                                start=(a == 0),
                                stop=(a == HEADS_PER_CORE - 1),
                            )
                    for tt in range(TT512):
                        store(tt)
                else:
                    # last dout: tt-major so each column block stores while the
                    # next one is still accumulating (shorter end tail)
                    for tt in range(TT512):
                        for a in range(HEADS_PER_CORE):
                            nc.tensor.matmul(
                                pss[tt][:],
                                lhsT=wo_sb[:, a, :],
                                rhs=at[a][:, tt * 512 : (tt + 1) * 512],
                                start=(a == 0),
                                stop=(a == HEADS_PER_CORE - 1),
                            )
                        store(tt)


# ======================= host-side sharding =======================


def _rope_tables(position_ids_b):
    pos = position_ids_b.astype(np.float32)  # [S]
    inv_freq = 1.0 / (ROPE_THETA ** (np.arange(0, HEAD_DIM, 2, dtype=np.float32) / HEAD_DIM))
    freqs = pos[:, None] * inv_freq[None, :]  # [S, 64]
    emb = np.concatenate([freqs, freqs], axis=1)  # [S, 128]
    cosT = np.ascontiguousarray(np.cos(emb).T.astype(np.float32))  # [128, S]
    sinT = np.ascontiguousarray(np.sin(emb).T.astype(np.float32))
    return cosT, sinT


def _shared_consts():
    rt = np.zeros((128, 128), dtype=ml_dtypes.bfloat16)
    idx = np.arange(64)
    rt[idx, idx + 64] = 1.0  # RT[j, j+64] = +1  (j < 64)
    rt[idx + 64, idx] = -1.0  # RT[j+64, j] = -1
    # pair masks: [128, 1024] where cols [0:512] use block alignment a and
    # cols [512:1024] use alignment a+1; mask[k, q] = (q >= a*128 + k)
    k = np.arange(128)[:, None]
    q = np.arange(512)[None, :]
    maskp = {}
    for a in (0, 2):
        m = np.zeros((128, 1024), dtype=ml_dtypes.bfloat16)
        m[:, 0:512] = (q >= a * 128 + k).astype(ml_dtypes.bfloat16)
        m[:, 512:1024] = (q >= (a + 1) * 128 + k).astype(ml_dtypes.bfloat16)
        maskp[a] = m
    return rt, maskp


def kernel(hidden_states, position_ids, Wq, Wk, Wv, Wo):
    bf16 = ml_dtypes.bfloat16
    if "nc" not in _BUILD_CACHE:
        _BUILD_CACHE["nc"] = _build_program()
    nc = _BUILD_CACHE["nc"]

    rt, maskp = _shared_consts()
    Wq16, Wk16, Wv16, Wo16 = (w.astype(bf16) for w in (Wq, Wk, Wv, Wo))

    xts, coss, sins = [], [], []
    for b in range(B):
        xb = np.asarray(hidden_states[b], dtype=np.float32).T.astype(bf16)  # [4096, S]
        xt = np.ascontiguousarray(xb.reshape(HC, 128, S).transpose(1, 0, 2))  # [128, 32, S]
        xts.append(xt)
        cosT, sinT = _rope_tables(np.asarray(position_ids[b]))
        coss.append(cosT)
        sins.append(sinT)

    in_maps = []
    for core in range(N_CORES):
        b, g = core // 4, core % 4
        wq = np.ascontiguousarray(
            Wq16[:, g * QDIM : (g + 1) * QDIM].reshape(HC, 128, HEADS_PER_CORE, 128).transpose(2, 1, 0, 3)
        )
        wk = np.ascontiguousarray(
            Wk16[:, g * KVDIM : (g + 1) * KVDIM].reshape(HC, 128, KV_PER_CORE, 128).transpose(2, 1, 0, 3)
        )
        wv = np.ascontiguousarray(
            Wv16[:, g * KVDIM : (g + 1) * KVDIM].reshape(HC, 128, KVDIM).transpose(1, 0, 2)
        )
        wo = np.ascontiguousarray(
            Wo16[g * QDIM : (g + 1) * QDIM, :].reshape(HEADS_PER_CORE, 128, HC, 128).transpose(2, 1, 0, 3)
        )
        in_maps.append(
            {
                "xt": xts[b],
                "wq": wq,
                "wk": wk,
                "wv": wv,
                "wo": wo,
                "cosT": coss[b],
                "sinT": sins[b],
                "rt": rt,
                "maskp0": maskp[0],
                "maskp2": maskp[2],
            }
        )

    res = run_bass_kernel_spmd(nc, in_maps, list(range(N_CORES))).results

    out = np.empty((B, S, HIDDEN), dtype=np.float32)
    for b in range(B):
        acc = res[4 * b]["out"].astype(np.float32)
        for g in range(1, 4):
            acc = acc + res[4 * b + g]["out"]
        out[b] = acc.T
    return out


# revision 13
# speedup vs baseline: 2.1694x; 1.6231x over previous
"""Trainium2 SPMD kernel for H2OLlama attention (GQA + RoPE + causal softmax + o_proj).

Sharding (8 cores): core = b*4 + g  (b in {0,1} batch, g in {0..3} head group).
Each core handles one batch element, 8 q-heads (g*8..g*8+8) and its 2 kv-heads,
computes QKV projections, RoPE, causal attention, and the row-sharded o_proj
partial product.  Host sums the 4 partials per batch (the o_proj all-reduce)
and transposes back.

All matmuls run in bf16 with fp32 PSUM accumulation; softmax runs in fp32 on
the Scalar engine (exp, no max subtraction -- scores are O(1) here).

Perf notes vs the first working version:
- softmax denominators for the large tiles accumulate on the Vector engine
  (bf16 pair tree) finished by two PSUM row-sum matmuls per (h, qt) instead of
  one ones-matmul per key block -- removes ~55us of PE column streaming; the
  small qt=0 tiles keep the per-block PSUM row-sum (engine balance).
- the 1/r broadcast matmul streams f32r (1 cycle/row) instead of f32 (4).
- scores/exp run on paired [128, 1024] PSUM tiles (half the Act instructions).
- x loads are chunked ([128, 8, 1024] x4) and DMA-issue order puts the first
  chunk + first weight ahead of the constants, so the PE starts ~6us in.
- the last o_proj dout runs column-major so stores overlap the final matmuls.
"""

import math
import sys

import numpy as np

sys.path.insert(0, "/opt/trn_rl_repo")

import ml_dtypes

import concourse.bass as bass
import concourse.tile as tile
from concourse import bacc, mybir
from concourse.bass_utils import run_bass_kernel_spmd

BF16 = mybir.dt.bfloat16
F32 = mybir.dt.float32
F32R = mybir.dt.float32r

HIDDEN = 4096
N_HEADS = 32
N_KV_HEADS = 8
HEAD_DIM = 128
B, S = 2, 2048
ROPE_THETA = 10000.0

N_CORES = 8
HEADS_PER_CORE = N_HEADS // 4  # 8 q heads per core (4 head groups)
KV_PER_CORE = N_KV_HEADS // 4  # 2 kv heads per core
QDIM = HEADS_PER_CORE * HEAD_DIM  # 1024
KVDIM = KV_PER_CORE * HEAD_DIM  # 256
HC = HIDDEN // 128  # 32 hidden chunks
TT512 = S // 512  # 4 token tiles of 512
TT128 = S // 128  # 16 token tiles of 128
SCALE = 1.0 / math.sqrt(HEAD_DIM)

_BUILD_CACHE = {}


def _build_program():
    nc = bacc.Bacc("TRN2", target_bir_lowering=False, debug=False, num_devices=N_CORES)

    # ---- DRAM I/O ----
    xt_d = nc.dram_tensor("xt", [128, HC, S], BF16, kind="ExternalInput")
    wq_d = nc.dram_tensor("wq", [HEADS_PER_CORE, 128, HC, 128], BF16, kind="ExternalInput")
    wk_d = nc.dram_tensor("wk", [KV_PER_CORE, 128, HC, 128], BF16, kind="ExternalInput")
    wv_d = nc.dram_tensor("wv", [128, HC, KVDIM], BF16, kind="ExternalInput")
    wo_d = nc.dram_tensor("wo", [HC, 128, HEADS_PER_CORE, 128], BF16, kind="ExternalInput")
    cos_d = nc.dram_tensor("cosT", [128, S], F32, kind="ExternalInput")
    sin_d = nc.dram_tensor("sinT", [128, S], F32, kind="ExternalInput")
    rt_d = nc.dram_tensor("rt", [128, 128], BF16, kind="ExternalInput")
    mp0_d = nc.dram_tensor("maskp0", [128, 1024], BF16, kind="ExternalInput")
    mp2_d = nc.dram_tensor("maskp2", [128, 1024], BF16, kind="ExternalInput")
    out_d = nc.dram_tensor("out", [HIDDEN, S], BF16, kind="ExternalOutput")

    with tile.TileContext(nc) as tc:
        _kernel_body(nc, tc, xt_d, wq_d, wk_d, wv_d, wo_d, cos_d, sin_d, rt_d, mp0_d, mp2_d, out_d)

    nc.compile()
    return nc


def _kernel_body(nc, tc, xt_d, wq_d, wk_d, wv_d, wo_d, cos_d, sin_d, rt_d, mp0_d, mp2_d, out_d):
    EXP = mybir.ActivationFunctionType.Exp

    with (
        tc.tile_pool(name="qr", bufs=1) as qr_pool,
        tc.tile_pool(name="kr", bufs=1) as kr_pool,
        tc.tile_pool(name="vv", bufs=1) as v_pool,
        tc.tile_pool(name="aa", bufs=1) as a_pool,
        tc.tile_pool(name="consts", bufs=1) as const_pool,
    ):
        # persistent tensors
        qr = [qr_pool.tile([128, S], BF16, tag=f"qr{h}", name=f"qr{h}") for h in range(HEADS_PER_CORE)]
        kr = [kr_pool.tile([128, S], BF16, tag=f"kr{k}", name=f"kr{k}") for k in range(KV_PER_CORE)]
        vt = v_pool.tile([128, TT128, KVDIM], BF16, tag="v", name="vt")
        at = [a_pool.tile([128, S], BF16, tag=f"a{h}", name=f"a{h}") for h in range(HEADS_PER_CORE)]

        cos_sb = const_pool.tile([128, S], F32, tag="cos", name="cos_sb")
        sin_sb = const_pool.tile([128, S], F32, tag="sin", name="sin_sb")
        rt_sb = const_pool.tile([128, 128], BF16, tag="rt", name="rt_sb")
        mp_sb = {
            0: const_pool.tile([128, 1024], BF16, tag="mp0", name="mp0_sb"),
            2: const_pool.tile([128, 1024], BF16, tag="mp2", name="mp2_sb"),
        }
        ones_row = const_pool.tile([1, 128], F32R, tag="ones_row", name="ones_row")

        onesf_row = const_pool.tile([1, 128], F32, tag="onesf_row", name="onesf_row")
        ones_colb = const_pool.tile([128, 1], BF16, tag="ones_colb", name="ones_colb")
        nc.vector.memset(onesf_row[:], 1.0)
        nc.vector.memset(ones_colb[:], 1.0)
        # memset cannot write f32r; convert through a DVE copy instead
        nc.vector.tensor_copy(ones_row[:], onesf_row[:])

        # ================= Stage 1+2: projections + RoPE =================
        with (
            tc.tile_pool(name="xt", bufs=1) as x_pool,
            tc.tile_pool(name="wqk", bufs=2) as wqk_pool,
            tc.tile_pool(name="wv", bufs=1) as wv_pool,
            tc.tile_pool(name="qkraw", bufs=1) as qkraw_pool,
            tc.tile_pool(name="ropetmp", bufs=1) as rtmp_pool,
            tc.tile_pool(name="qkps", bufs=2, space="PSUM") as qk_psum,
            tc.tile_pool(name="rotps", bufs=1, space="PSUM") as rot_psum,
            tc.tile_pool(name="vps", bufs=2, space="PSUM") as v_psum,
        ):
            wv_sb = wv_pool.tile([128, HC, KVDIM], BF16, tag="wv", name="wv_sb")
            for half in range(2):
                toff = half * 1024
                # x chunks for this half: 4 tiles of 8 hidden-chunks each
                xs = []
                for cg in range(4):
                    xtile = x_pool.tile([128, 8, 1024], BF16, tag=f"x{cg}", name=f"x{cg}")
                    xs.append(xtile)

                def xdma(cg, n_sub=2, eng=None):
                    # sub-DMAs per chunk so downstream matmuls can start on
                    # the first hidden-chunks while the rest streams in
                    step = 8 // n_sub
                    for s in range(n_sub):
                        lo, hi = s * step, (s + 1) * step
                        (eng or nc.sync).dma_start(
                            xs[cg][:, lo:hi, :],
                            xt_d.ap()[:, cg * 8 + lo : cg * 8 + hi, toff : toff + 1024],
                        )

                def xsl(hc, lo, sz):
                    return xs[hc // 8][:, hc % 8, lo : lo + sz]

                w_tiles = {}

                def wdma(do, split=False):
                    if do < HEADS_PER_CORE:
                        w_src = wq_d.ap()[do]
                    else:
                        w_src = wk_d.ap()[do - HEADS_PER_CORE]
                    w_sb = wqk_pool.tile([128, HC, 128], BF16, tag="wqk", name="w_sb")
                    if split:
                        # sub-DMAs so the first matmuls start before the whole
                        # weight tile lands
                        for s in range(4):
                            nc.sync.dma_start(
                                w_sb[:, s * 8 : (s + 1) * 8, :], w_src[:, s * 8 : (s + 1) * 8, :]
                            )
                    else:
                        nc.sync.dma_start(w_sb[:], w_src)
                    w_tiles[do] = w_sb

                # DMA issue order: first weight + first x chunk interleaved at
                # fine grain ahead of the constants so the PE starts ~3us in;
                # constants are only needed by RoPE (DVE) / V-proj, much later.
                if half == 0:
                    # interleave the first weight's pieces with the first x
                    # pieces so the very first matmuls have both inputs early
                    w_sb0 = wqk_pool.tile([128, HC, 128], BF16, tag="wqk", name="w_sb")
                    w_src0 = wq_d.ap()[0]
                    for s in range(4):
                        nc.sync.dma_start(
                            w_sb0[:, s * 8 : (s + 1) * 8, :], w_src0[:, s * 8 : (s + 1) * 8, :]
                        )
                        nc.sync.dma_start(
                            xs[0][:, s * 2 : (s + 1) * 2, :],
                            xt_d.ap()[:, s * 2 : (s + 1) * 2, toff : toff + 1024],
                        )
                    w_tiles[0] = w_sb0
                else:
                    xdma(0, n_sub=4)
                    wdma(0)
                xdma(1)
                xdma(2)
                xdma(3)
                wdma(1)
                if half == 0:
                    nc.sync.dma_start(rt_sb[:], rt_d.ap())
                    nc.sync.dma_start(cos_sb[:], cos_d.ap())
                    nc.sync.dma_start(sin_sb[:], sin_d.ap())
                    nc.sync.dma_start(mp_sb[0][:], mp0_d.ap())
                    nc.sync.dma_start(mp_sb[2][:], mp2_d.ap())
                    nc.sync.dma_start(wv_sb[:], wv_d.ap())

                # ---- Q^T and K^T projections (+ RoPE) ----
                for do in range(HEADS_PER_CORE + KV_PER_CORE):  # 8 q douts, 2 k douts
                    if do + 2 < HEADS_PER_CORE + KV_PER_CORE:
                        wdma(do + 2)
                    if do < HEADS_PER_CORE:
                        dst = qr[do]
                    else:
                        dst = kr[do - HEADS_PER_CORE]
                    w_sb = w_tiles.pop(do)

                    ps = qk_psum.tile([128, 1024], F32, tag="qk", name="qkps")
                    for hc in range(HC):
                        for tt in range(2):
                            nc.tensor.matmul(
                                ps[:, tt * 512 : (tt + 1) * 512],
                                lhsT=w_sb[:, hc, :],
                                rhs=xsl(hc, tt * 512, 512),
                                start=(hc == 0),
                                stop=(hc == HC - 1),
                            )
                    # RoPE on the full 1024-token tile
                    raw = qkraw_pool.tile([128, 1024], BF16, tag="raw", name="raw")
                    nc.scalar.copy(raw[:], ps[:])
                    rot = rot_psum.tile([128, 1024], F32, tag="rot", name="rot")
                    for tt in range(2):
                        nc.tensor.matmul(
                            rot[:, tt * 512 : (tt + 1) * 512],
                            lhsT=rt_sb[:],
                            rhs=raw[:, tt * 512 : (tt + 1) * 512],
                            start=True,
                            stop=True,
                        )
                    t1 = rtmp_pool.tile([128, 1024], F32, tag="t1", name="t1")
                    nc.vector.tensor_mul(t1[:], raw[:], cos_sb[:, toff : toff + 1024])
                    t2 = rtmp_pool.tile([128, 1024], F32, tag="t2", name="t2")
                    nc.vector.tensor_mul(t2[:], rot[:], sin_sb[:, toff : toff + 1024])
                    nc.vector.tensor_add(dst[:, toff : toff + 1024], t1[:], t2[:])

                # ---- V projection (natural [t, d] layout) ----
                for t8 in range(8):
                    j = half * 8 + t8
                    ps = v_psum.tile([128, KVDIM], F32, tag="vps", name="vps")
                    for hc in range(HC):
                        nc.tensor.matmul(
                            ps[:],
                            lhsT=xsl(hc, t8 * 128, 128),
                            rhs=wv_sb[:, hc, :],
                            start=(hc == 0),
                            stop=(hc == HC - 1),
                        )
                    nc.scalar.copy(vt[:, j, :], ps[:])

        # ================= Stage 3: attention =================
        with (
            tc.tile_pool(name="pp", bufs=6) as p_pool,
            tc.tile_pool(name="acc", bufs=2) as acc_pool,
            tc.tile_pool(name="rinv", bufs=2) as rinv_pool,
            tc.tile_pool(name="sps", bufs=2, space="PSUM") as s_psum,
            tc.tile_pool(name="ops", bufs=2, space="PSUM") as o_psum,
            tc.tile_pool(name="rps", bufs=1, space="PSUM") as r_psum,
            tc.tile_pool(name="bps", bufs=1, space="PSUM") as b_psum,
        ):
            for h in range(HEADS_PER_CORE):
                kvl = h // 4
                for qt in range(TT512):
                    nj = 4 * qt + 4
                    npair = nj // 2
                    # small tiles keep the baseline PE-PSUM row-sum; large tiles
                    # accumulate denominators on the DVE to unload the PE.  The
                    # split keeps PE, DVE, and Act roughly level in this stage.
                    dve_acc = True
                    q_rhs = qr[h][:, qt * 512 : (qt + 1) * 512]
                    o_ps = o_psum.tile([128, 512], F32, tag="o", name="o_ps")
                    r_ps = r_psum.tile([1, 512], F32, tag="r", name="r_ps")
                    if dve_acc:
                        acc2 = acc_pool.tile([128, 1024], BF16, tag="acc2", name="acc2")

                    pts = {}

                    def emit_pair(pi):
                        j0 = 2 * pi
                        s_ps = s_psum.tile([128, 1024], F32, tag="s", name="s_ps")
                        trimmed = dve_acc and j0 - 4 * qt == 2
                        if trimmed:
                            # last diagonal pair (blocks a=2,3): queries below
                            # the causal boundary are fully masked -- compute
                            # only the valid column ranges [256:512] / [896:1024]
                            nc.tensor.matmul(
                                s_ps[:, 256:512],
                                lhsT=kr[kvl][:, j0 * 128 : (j0 + 1) * 128],
                                rhs=q_rhs[:, 256:512],
                                start=True,
                                stop=True,
                            )
                            nc.tensor.matmul(
                                s_ps[:, 896:1024],
                                lhsT=kr[kvl][:, (j0 + 1) * 128 : (j0 + 2) * 128],
                                rhs=q_rhs[:, 384:512],
                                start=True,
                                stop=True,
                            )
                            p2 = p_pool.tile([128, 1024], BF16, tag="p", name="p2")
                            nc.scalar.activation(p2[:, 256:512], s_ps[:, 256:512], EXP, scale=SCALE)
                            nc.scalar.activation(p2[:, 896:1024], s_ps[:, 896:1024], EXP, scale=SCALE)
                            nc.vector.tensor_mul(p2[:, 256:512], p2[:, 256:512], mp_sb[2][:, 256:512])
                            nc.vector.tensor_mul(p2[:, 896:1024], p2[:, 896:1024], mp_sb[2][:, 896:1024])
                            pts[pi] = p2
                            return
                        for u in range(2):
                            nc.tensor.matmul(
                                s_ps[:, u * 512 : (u + 1) * 512],
                                lhsT=kr[kvl][:, (j0 + u) * 128 : (j0 + u + 1) * 128],
                                rhs=q_rhs,
                                start=True,
                                stop=True,
                            )
                        p2 = p_pool.tile([128, 1024], BF16, tag="p", name="p2")
                        nc.scalar.activation(p2[:], s_ps[:], EXP, scale=SCALE)
                        if j0 >= 4 * qt:  # diagonal pair: multiplicative causal mask
                            a = j0 - 4 * qt  # 0 or 2
                            nc.vector.tensor_mul(p2[:], p2[:], mp_sb[a][:])
                        pts[pi] = p2

                    # software pipeline: score pairs run 2 iterations ahead of AV
                    emit_pair(0)
                    if npair > 1:
                        emit_pair(1)
                    for pi in range(npair):
                        if pi + 2 < npair:
                            emit_pair(pi + 2)
                        p2 = pts.pop(pi)
                        trimmed = dve_acc and 2 * pi - 4 * qt == 2
                        if dve_acc:
                            # denominator accumulation on DVE (pair-wide bf16 tree)
                            if pi == 0:
                                nc.vector.tensor_copy(acc2[:], p2[:])
                            elif trimmed:
                                nc.vector.tensor_add(acc2[:, 256:512], acc2[:, 256:512], p2[:, 256:512])
                                nc.vector.tensor_add(acc2[:, 896:1024], acc2[:, 896:1024], p2[:, 896:1024])
                            else:
                                nc.vector.tensor_add(acc2[:], acc2[:], p2[:])
                        for u in range(2):
                            j = 2 * pi + u
                            a = j - 4 * qt
                            if dve_acc and a >= 1:
                                # column-range accumulation stops: cols [0:256]
                                # finish at a=1, [256:384] at a=2, [384:512] at a=3
                                vt_l = vt[:, j, kvl * 128 : (kvl + 1) * 128]
                                if a == 1:
                                    nc.tensor.matmul(o_ps[:, 0:256], lhsT=vt_l,
                                                     rhs=p2[:, 512:768], start=False, stop=True,
                                                     skip_group_check=True)
                                    nc.tensor.matmul(o_ps[:, 256:512], lhsT=vt_l,
                                                     rhs=p2[:, 768:1024], start=False, stop=False,
                                                     skip_group_check=True)
                                elif a == 2:
                                    nc.tensor.matmul(o_ps[:, 256:384], lhsT=vt_l,
                                                     rhs=p2[:, 256:384], start=False, stop=True,
                                                     skip_group_check=True)
                                    nc.tensor.matmul(o_ps[:, 384:512], lhsT=vt_l,
                                                     rhs=p2[:, 384:512], start=False, stop=False,
                                                     skip_group_check=True)
                                else:  # a == 3
                                    nc.tensor.matmul(o_ps[:, 384:512], lhsT=vt_l,
                                                     rhs=p2[:, 896:1024], start=False, stop=True,
                                                     skip_group_check=True)
                                continue
                            nc.tensor.matmul(
                                o_ps[:],
                                lhsT=vt[:, j, kvl * 128 : (kvl + 1) * 128],
                                rhs=p2[:, u * 512 : (u + 1) * 512],
                                start=(j == 0),
                                stop=(not dve_acc and j == nj - 1),
                                skip_group_check=dve_acc,
                            )
                            if not dve_acc:
                                nc.tensor.matmul(
                                    r_ps[:],
                                    lhsT=ones_colb[:],
                                    rhs=p2[:, u * 512 : (u + 1) * 512],
                                    start=(j == 0),
                                    stop=(j == nj - 1),
                                )
                    if dve_acc:
                        # row-sum both acc2 halves directly on PE (bf16,
                        # PSUM-accumulated) -- no DVE fold on the critical path
                        for u in range(2):
                            nc.tensor.matmul(
                                r_ps[:],
                                lhsT=ones_colb[:],
                                rhs=acc2[:, u * 512 : (u + 1) * 512],
                                start=(u == 0),
                                stop=(u == 1),
                            )
                    rinv = rinv_pool.tile([1, 512], F32R, tag="rinv", name="rinv")
                    with nc.allow_low_precision(reason="f32r reciprocal feeds the f32r broadcast matmul; ~10-bit mantissa is ample for softmax denominators"):
                        nc.vector.reciprocal(rinv[:], r_ps[:])
                    b_ps = b_psum.tile([128, 512], F32, tag="b", name="b_ps")
                    nc.tensor.matmul(
                        b_ps[:],
                        lhsT=ones_row[:],
                        rhs=rinv[:],
                        start=True,
                        stop=True,
                    )
                    # stage the broadcast through SBUF so the normalize multiply
                    # only touches one PSUM operand; alternate the copy engine
                    # to keep Act and DVE level (both near-saturated here)
                    b_sb = rinv_pool.tile([128, 512], F32, tag="bsb", name="b_sb")
                    if qt % 2 == 1:
                        nc.scalar.copy(b_sb[:], b_ps[:])
                    else:
                        nc.vector.tensor_copy(b_sb[:], b_ps[:])
                    nc.vector.tensor_mul(at[h][:, qt * 512 : (qt + 1) * 512], o_ps[:], b_sb[:])

        # ================= Stage 4: o_proj (out^T layout) =================
        with (
            tc.tile_pool(name="wo", bufs=2) as wo_pool,
            tc.tile_pool(name="oout", bufs=4) as out_pool,
            tc.tile_pool(name="outps", bufs=2, space="PSUM") as out_psum,
        ):
            for do in range(HC):  # 32 dout tiles of 128
                wo_sb = wo_pool.tile([128, HEADS_PER_CORE, 128], BF16, tag="wo", name="wo_sb")
                nc.sync.dma_start(wo_sb[:], wo_d.ap()[do])
                pss = [out_psum.tile([128, 512], F32, tag=f"op{tt}", name=f"op{tt}") for tt in range(TT512)]

                def store(tt):
                    ot = out_pool.tile([128, 512], BF16, tag="ot", name="ot")
                    nc.vector.tensor_copy(ot[:], pss[tt][:])
                    nc.sync.dma_start(
                        out_d.ap()[do * 128 : (do + 1) * 128, tt * 512 : (tt + 1) * 512], ot[:]
                    )

                if do < HC - 1:
                    for a in range(HEADS_PER_CORE):
                        for tt in range(TT512):
                            nc.tensor.matmul(
                                pss[tt][:],
                                lhsT=wo_sb[:, a, :],
                                rhs=at[a][:, tt * 512 : (tt + 1) * 512],
                                start=(a == 0),
                                stop=(a == HEADS_PER_CORE - 1),
                            )
                    for tt in range(TT512):
                        store(tt)
                else:
                    # last dout: tt-major so each column block stores while the
                    # next one is still accumulating (shorter end tail)
                    for tt in range(TT512):
                        for a in range(HEADS_PER_CORE):
                            nc.tensor.matmul(
                                pss[tt][:],
                                lhsT=wo_sb[:, a, :],
                                rhs=at[a][:, tt * 512 : (tt + 1) * 512],
                                start=(a == 0),
                                stop=(a == HEADS_PER_CORE - 1),
                            )
                        store(tt)


# ======================= host-side sharding =======================


def _rope_tables(position_ids_b):
    pos = position_ids_b.astype(np.float32)  # [S]
    inv_freq = 1.0 / (ROPE_THETA ** (np.arange(0, HEAD_DIM, 2, dtype=np.float32) / HEAD_DIM))
    freqs = pos[:, None] * inv_freq[None, :]  # [S, 64]
    emb = np.concatenate([freqs, freqs], axis=1)  # [S, 128]
    cosT = np.ascontiguousarray(np.cos(emb).T.astype(np.float32))  # [128, S]
    sinT = np.ascontiguousarray(np.sin(emb).T.astype(np.float32))
    return cosT, sinT


def _shared_consts():
    rt = np.zeros((128, 128), dtype=ml_dtypes.bfloat16)
    idx = np.arange(64)
    rt[idx, idx + 64] = 1.0  # RT[j, j+64] = +1  (j < 64)
    rt[idx + 64, idx] = -1.0  # RT[j+64, j] = -1
    # pair masks: [128, 1024] where cols [0:512] use block alignment a and
    # cols [512:1024] use alignment a+1; mask[k, q] = (q >= a*128 + k)
    k = np.arange(128)[:, None]
    q = np.arange(512)[None, :]
    maskp = {}
    for a in (0, 2):
        m = np.zeros((128, 1024), dtype=ml_dtypes.bfloat16)
        m[:, 0:512] = (q >= a * 128 + k).astype(ml_dtypes.bfloat16)
        m[:, 512:1024] = (q >= (a + 1) * 128 + k).astype(ml_dtypes.bfloat16)
        maskp[a] = m
    return rt, maskp


def kernel(hidden_states, position_ids, Wq, Wk, Wv, Wo):
    bf16 = ml_dtypes.bfloat16
    if "nc" not in _BUILD_CACHE:
        _BUILD_CACHE["nc"] = _build_program()
    nc = _BUILD_CACHE["nc"]

    rt, maskp = _shared_consts()
    Wq16, Wk16, Wv16, Wo16 = (w.astype(bf16) for w in (Wq, Wk, Wv, Wo))

    xts, coss, sins = [], [], []
    for b in range(B):
        xb = np.asarray(hidden_states[b], dtype=np.float32).T.astype(bf16)  # [4096, S]
        xt = np.ascontiguousarray(xb.reshape(HC, 128, S).transpose(1, 0, 2))  # [128, 32, S]
        xts.append(xt)
        cosT, sinT = _rope_tables(np.asarray(position_ids[b]))
        coss.append(cosT)
        sins.append(sinT)

    in_maps = []
    for core in range(N_CORES):
        b, g = core // 4, core % 4
        wq = np.ascontiguousarray(
            Wq16[:, g * QDIM : (g + 1) * QDIM].reshape(HC, 128, HEADS_PER_CORE, 128).transpose(2, 1, 0, 3)
        )
        wk = np.ascontiguousarray(
            Wk16[:, g * KVDIM : (g + 1) * KVDIM].reshape(HC, 128, KV_PER_CORE, 128).transpose(2, 1, 0, 3)
        )
        wv = np.ascontiguousarray(
            Wv16[:, g * KVDIM : (g + 1) * KVDIM].reshape(HC, 128, KVDIM).transpose(1, 0, 2)
        )
        wo = np.ascontiguousarray(
            Wo16[g * QDIM : (g + 1) * QDIM, :].reshape(HEADS_PER_CORE, 128, HC, 128).transpose(2, 1, 0, 3)
        )
        in_maps.append(
            {
                "xt": xts[b],
                "wq": wq,
                "wk": wk,
                "wv": wv,
                "wo": wo,
                "cosT": coss[b],
                "sinT": sins[b],
                "rt": rt,
                "maskp0": maskp[0],
                "maskp2": maskp[2],
            }
        )

    res = run_bass_kernel_spmd(nc, in_maps, list(range(N_CORES))).results

    out = np.empty((B, S, HIDDEN), dtype=np.float32)
    for b in range(B):
        acc = res[4 * b]["out"].astype(np.float32)
        for g in range(1, 4):
            acc = acc + res[4 * b + g]["out"]
        out[b] = acc.T
    return out
